# revision 2
# baseline (speedup 1.0000x reference)
"""PointerHead kernel for 8x TRN2 NeuronCores.

Reference computes, per batch row b with X = path_features[b] (3 x 2048):
    qkv = X @ W.T + b_             (3 x 6144), split into Q,K,V (3 x 2048)
    S   = Q @ K.T / sqrt(2048)     (3 x 3)
    A   = softmax(S, axis=-1)
    out = (A @ V).sum(-1)          (3,)

Algebraic reductions used here (all exact in real arithmetic):
  * S = X Ms X^T + 1 qs^T (+ row-const, dropped by softmax) with
    Ms = (Wq^T Wk)/sqrt(d), qs_j = X_j . (Wk^T bq)/sqrt(d).
    The qs term is folded into the matmul as one extra contraction row:
    Y = [X | 1] @ [[Ms], [ms]] with ms = (Wk^T bq)/sqrt(d), so
    S_ij = Y_i . X_j directly.
  * out_i = sum_j A_ij * (X_j . colsum(Wv)) + sum(bv); the constant is
    added on host (softmax rows sum to 1), V is never materialized.
Device work per core (1024 batches): one (3072x2048)@(2048x2048) bf16
matmul (Y) + DVE row-dot grams (tensor_mul + reduce_sum; the fused
tensor_tensor_reduce crashes the exec unit on this HW) + tiny softmax.
"""

import math
import sys

import numpy as np

if "/opt/trn_rl_repo" not in sys.path:
    sys.path.insert(0, "/opt/trn_rl_repo")

import ml_dtypes

BF16 = ml_dtypes.bfloat16

D = 2048
B = 8192
NCORES = 8
PB = B // NCORES        # batches per core
G = PB // 128           # groups of 128 batches per core
NBLK = 3 * G            # row blocks per core; blk = g*3 + i
DC = D // 128           # contraction chunks
SCALE = 1.0 / math.sqrt(D)

_CACHE = {}
LAST_RESULTS = None


def _build():
    if "nc" in _CACHE:
        return _CACHE["nc"]

    import concourse.bass as bass
    import concourse.tile as tile
    from concourse import bacc, mybir

    bf = mybir.dt.bfloat16
    f32 = mybir.dt.float32

    nc = bacc.Bacc("TRN2", target_bir_lowering=False, debug=False,
                   num_devices=NCORES)

    xt_d = nc.dram_tensor("xt", [NBLK, DC, 128, 128], bf, kind="ExternalInput")
    xn_d = nc.dram_tensor("xn", [NBLK, 128, D], bf, kind="ExternalInput")
    mm_d = nc.dram_tensor("mm", [DC, 128, D], bf, kind="ExternalInput")
    mr_d = nc.dram_tensor("mrow", [1, D], bf, kind="ExternalInput")
    on_d = nc.dram_tensor("ones", [1, 128], bf, kind="ExternalInput")
    wr_d = nc.dram_tensor("wvrep", [128, D], bf, kind="ExternalInput")
    out_d = nc.dram_tensor("out", [G, 128, 3], f32, kind="ExternalOutput")

    AX = mybir.AxisListType
    AF = mybir.ActivationFunctionType

    with tile.TileContext(nc) as tc:
        with (
            tc.tile_pool(name="wpool", bufs=1) as wpool,
            tc.tile_pool(name="xtp", bufs=3) as xtp,
            tc.tile_pool(name="xnp", bufs=2) as xnp,
            tc.tile_pool(name="ypool", bufs=2) as ypool,
            tc.tile_pool(name="prodp", bufs=2) as prodp,
            tc.tile_pool(name="small", bufs=3) as small,
            tc.tile_pool(name="psum", bufs=2, space="PSUM") as psum,
        ):
            mm_s = wpool.tile([128, DC, D], bf)
            for dc in range(DC):
                nc.sync.dma_start(mm_s[:, dc, :], mm_d[dc])
            mr_s = wpool.tile([1, D], bf)
            on_s = wpool.tile([1, 128], bf)
            wr_s = wpool.tile([128, D], bf)
            nc.gpsimd.dma_start(mr_s[:], mr_d[:])
            nc.gpsimd.dma_start(on_s[:], on_d[:])
            nc.gpsimd.dma_start(wr_s[:], wr_d[:])

            for g in range(G):
                xn3 = xnp.tile([128, 3, D], bf)
                for j in range(3):
                    nc.gpsimd.dma_start(xn3[:, j, :], xn_d[g * 3 + j])

                # vs_j = X_j . colsum(Wv) for the 128 batches of this group
                vs = small.tile([128, 3], f32)
                prod = prodp.tile([128, D], f32)
                for j in range(3):
                    nc.vector.tensor_mul(prod[:], xn3[:, j, :], wr_s[:])
                    nc.vector.reduce_sum(vs[:, j:j + 1], prod[:], axis=AX.X)

                # Y_i = [X_i | 1] @ [[Ms],[ms]] on PE, grams on DVE
                y3 = ypool.tile([128, 3, D], bf)
                for i in range(3):
                    blk = g * 3 + i
                    xts = xtp.tile([128, DC, 128], bf)
                    for dc in range(DC):
                        nc.sync.dma_start(xts[:, dc, :], xt_d[blk, dc])
                    py = psum.tile([128, D], f32)
                    for dc in range(DC):
                        for ec in range(4):
                            nc.tensor.matmul(
                                py[:, ec * 512:(ec + 1) * 512],
                                xts[:, dc, :],
                                mm_s[:, dc, ec * 512:(ec + 1) * 512],
                                start=(dc == 0), stop=False)
                    for ec in range(4):
                        nc.tensor.matmul(
                            py[:, ec * 512:(ec + 1) * 512],
                            on_s[:],
                            mr_s[:, ec * 512:(ec + 1) * 512],
                            start=False, stop=True)
                    nc.scalar.copy(y3[:, i, :], py[:])

                # S[:, i*3+j] = Y_i . X_j  (scale already folded into Ms/ms)
                S = small.tile([128, 9], f32)
                for i in range(3):
                    for j in range(3):
                        nc.vector.tensor_mul(prod[:], y3[:, i, :], xn3[:, j, :])
                        nc.vector.reduce_sum(S[:, 3 * i + j:3 * i + j + 1],
                                             prod[:], axis=AX.X)

                E = small.tile([128, 9], f32)
                nc.scalar.activation(E[:], S[:], AF.Exp)

                den = small.tile([128, 3], f32)
                num = small.tile([128, 3], f32)
                prod3 = small.tile([128, 3], f32)
                for i in range(3):
                    nc.vector.reduce_sum(den[:, i:i + 1], E[:, 3 * i:3 * i + 3],
                                         axis=AX.X)
                    nc.vector.tensor_mul(prod3[:], E[:, 3 * i:3 * i + 3], vs[:])
                    nc.vector.reduce_sum(num[:, i:i + 1], prod3[:], axis=AX.X)
                rden = small.tile([128, 3], f32)
                nc.vector.reciprocal(rden[:], den[:])
                logit = small.tile([128, 3], f32)
                nc.vector.tensor_mul(logit[:], num[:], rden[:])
                nc.gpsimd.dma_start(out_d[g], logit[:])

    nc.compile()
    _CACHE["nc"] = nc
    return nc


def kernel(path_features, W, b):
    global LAST_RESULTS
    from concourse.bass_utils import run_bass_kernel_spmd

    pf = np.ascontiguousarray(path_features, dtype=np.float32)
    W = np.asarray(W, dtype=np.float32)
    bias = np.asarray(b, dtype=np.float32)
    Wq, Wk, Wv = W[0:D], W[D:2 * D], W[2 * D:3 * D]
    bq, bv = bias[0:D], bias[2 * D:3 * D]

    Ms = (Wq.T @ Wk) * SCALE                        # (D, D) fp32
    mm = np.ascontiguousarray(Ms.reshape(DC, 128, D)).astype(BF16)
    ms = ((Wk.T @ bq) * SCALE).astype(BF16).reshape(1, D)
    ones = np.ones((1, 128), dtype=BF16)
    wv = Wv.sum(axis=0)                             # (D,)
    wvrep = np.ascontiguousarray(np.broadcast_to(wv.astype(BF16), (128, D)))
    c1 = float(bv.sum())

    nc = _build()

    in_maps = []
    for c in range(NCORES):
        Xc = pf[c * PB:(c + 1) * PB]                # (PB, 3, D)
        Xr = Xc.reshape(G, 128, 3, D).transpose(0, 2, 1, 3).reshape(NBLK, 128, D)
        xn = Xr.astype(BF16)
        xt = np.ascontiguousarray(
            Xr.reshape(NBLK, 128, DC, 128).transpose(0, 2, 3, 1)).astype(BF16)
        in_maps.append({"xt": xt, "xn": xn, "mm": mm, "mrow": ms,
                        "ones": ones, "wvrep": wvrep})

    res = run_bass_kernel_spmd(nc, in_maps, core_ids=list(range(NCORES)))
    LAST_RESULTS = res

    out = np.concatenate(
        [res.results[c]["out"].reshape(PB, 3) for c in range(NCORES)], axis=0)
    return (out + c1).astype(np.float32)


# revision 4
# speedup vs baseline: 11680.3694x; 11680.3694x over previous
"""PointerHead kernel for 8x TRN2 NeuronCores.

Reference computes, per batch row b with X = path_features[b] (3 x 2048):
    qkv = X @ W.T + b_             (3 x 6144), split into Q,K,V (3 x 2048)
    S   = Q @ K.T / sqrt(2048)     (3 x 3)
    A   = softmax(S, axis=-1)
    out = (A @ V).sum(-1)          (3,)

Algebraic reductions used here (all exact in real arithmetic):
  * S = X Ms X^T + 1 qs^T (+ row-const, dropped by softmax) with
    Ms = (Wq^T Wk)/sqrt(d), qs_j = X_j . (Wk^T bq)/sqrt(d).
    The qs term is folded into the matmul as one extra contraction row:
    Y = [X | 1] @ [[Ms], [ms]] with ms = (Wk^T bq)/sqrt(d), so
    S_ij = Y_i . X_j directly.
  * out_i = sum_j A_ij * (X_j . colsum(Wv)) + sum(bv); the constant is
    added on host (softmax rows sum to 1), V is never materialized.
Device work per core (1024 batches): one (3072x2048)@(2048x2048) bf16
matmul (Y) + DVE row-dot grams (tensor_mul + reduce_sum; the fused
tensor_tensor_reduce crashes the exec unit on this HW) + tiny softmax.
"""

import math
import os
import sys

import numpy as np

if "/opt/trn_rl_repo" not in sys.path:
    sys.path.insert(0, "/opt/trn_rl_repo")

import ml_dtypes

BF16 = ml_dtypes.bfloat16

D = 2048
B = 8192
NCORES = 8
PB = B // NCORES        # batches per core
G = PB // 128           # groups of 128 batches per core
NBLK = 3 * G            # row blocks per core; blk = g*3 + i
DC = D // 128           # contraction chunks
SCALE = 1.0 / math.sqrt(D)

_CACHE = {}
LAST_RESULTS = None


def _build():
    if "nc" in _CACHE:
        return _CACHE["nc"]

    import concourse.bass as bass
    import concourse.tile as tile
    from concourse import bacc, mybir

    bf = mybir.dt.bfloat16
    f32 = mybir.dt.float32

    nc = bacc.Bacc("TRN2", target_bir_lowering=False, debug=False,
                   num_devices=NCORES)

    xt_d = nc.dram_tensor("xt", [NBLK, DC, 128, 128], bf, kind="ExternalInput")
    xn_d = nc.dram_tensor("xn", [NBLK, 128, D], bf, kind="ExternalInput")
    mm_d = nc.dram_tensor("mm", [DC, 128, D], bf, kind="ExternalInput")
    mr_d = nc.dram_tensor("mrow", [1, D], bf, kind="ExternalInput")
    on_d = nc.dram_tensor("ones", [1, 128], bf, kind="ExternalInput")
    wr_d = nc.dram_tensor("wvrep", [128, D], bf, kind="ExternalInput")
    out_d = nc.dram_tensor("out", [G, 128, 3], f32, kind="ExternalOutput")

    AX = mybir.AxisListType
    AF = mybir.ActivationFunctionType

    with tile.TileContext(nc) as tc:
        with (
            tc.tile_pool(name="wpool", bufs=1) as wpool,
            tc.tile_pool(name="xtp", bufs=3) as xtp,
            tc.tile_pool(name="xnp", bufs=2) as xnp,
            tc.tile_pool(name="ypool", bufs=2) as ypool,
            tc.tile_pool(name="prodp", bufs=2) as prodp,
            tc.tile_pool(name="small", bufs=3) as small,
            tc.tile_pool(name="psum", bufs=2, space="PSUM") as psum,
        ):
            mm_s = wpool.tile([128, DC, D], bf)
            for dc in range(DC):
                nc.sync.dma_start(mm_s[:, dc, :], mm_d[dc])
            mr_s = wpool.tile([1, D], bf)
            on_s = wpool.tile([1, 128], bf)
            wr_s = wpool.tile([128, D], bf)
            nc.gpsimd.dma_start(mr_s[:], mr_d[:])
            nc.gpsimd.dma_start(on_s[:], on_d[:])
            nc.gpsimd.dma_start(wr_s[:], wr_d[:])

            for g in range(G):
                xn3 = xnp.tile([128, 3, D], bf)
                for j in range(3):
                    nc.gpsimd.dma_start(xn3[:, j, :], xn_d[g * 3 + j])

                # vs_j = X_j . colsum(Wv) for the 128 batches of this group
                vs = small.tile([128, 3], f32)
                prod = prodp.tile([128, D], f32)
                for j in range(3):
                    nc.vector.tensor_mul(prod[:], xn3[:, j, :], wr_s[:])
                    nc.vector.reduce_sum(vs[:, j:j + 1], prod[:], axis=AX.X)

                # Y_i = [X_i | 1] @ [[Ms],[ms]] on PE, grams on DVE
                y3 = ypool.tile([128, 3, D], bf)
                for i in range(3):
                    blk = g * 3 + i
                    xts = xtp.tile([128, DC, 128], bf)
                    for dc in range(DC):
                        nc.sync.dma_start(xts[:, dc, :], xt_d[blk, dc])
                    py = psum.tile([128, D], f32)
                    for dc in range(DC):
                        for ec in range(4):
                            nc.tensor.matmul(
                                py[:, ec * 512:(ec + 1) * 512],
                                xts[:, dc, :],
                                mm_s[:, dc, ec * 512:(ec + 1) * 512],
                                start=(dc == 0), stop=False)
                    for ec in range(4):
                        nc.tensor.matmul(
                            py[:, ec * 512:(ec + 1) * 512],
                            on_s[:],
                            mr_s[:, ec * 512:(ec + 1) * 512],
                            start=False, stop=True)
                    nc.scalar.copy(y3[:, i, :], py[:])

                # S[:, i*3+j] = Y_i . X_j  (scale already folded into Ms/ms)
                S = small.tile([128, 9], f32)
                for i in range(3):
                    for j in range(3):
                        nc.vector.tensor_mul(prod[:], y3[:, i, :], xn3[:, j, :])
                        nc.vector.reduce_sum(S[:, 3 * i + j:3 * i + j + 1],
                                             prod[:], axis=AX.X)

                E = small.tile([128, 9], f32)
                nc.scalar.activation(E[:], S[:], AF.Exp)

                den = small.tile([128, 3], f32)
                num = small.tile([128, 3], f32)
                prod3 = small.tile([128, 3], f32)
                for i in range(3):
                    nc.vector.reduce_sum(den[:, i:i + 1], E[:, 3 * i:3 * i + 3],
                                         axis=AX.X)
                    nc.vector.tensor_mul(prod3[:], E[:, 3 * i:3 * i + 3], vs[:])
                    nc.vector.reduce_sum(num[:, i:i + 1], prod3[:], axis=AX.X)
                rden = small.tile([128, 3], f32)
                nc.vector.reciprocal(rden[:], den[:])
                logit = small.tile([128, 3], f32)
                nc.vector.tensor_mul(logit[:], num[:], rden[:])
                nc.gpsimd.dma_start(out_d[g], logit[:])

    nc.compile()
    _CACHE["nc"] = nc
    return nc


def kernel(path_features, W, b):
    global LAST_RESULTS
    from concourse.bass_utils import run_bass_kernel_spmd

    pf = np.ascontiguousarray(path_features, dtype=np.float32)
    W = np.asarray(W, dtype=np.float32)
    bias = np.asarray(b, dtype=np.float32)
    Wq, Wk, Wv = W[0:D], W[D:2 * D], W[2 * D:3 * D]
    bq, bv = bias[0:D], bias[2 * D:3 * D]

    Ms = (Wq.T @ Wk) * SCALE                        # (D, D) fp32
    mm = np.ascontiguousarray(Ms.reshape(DC, 128, D)).astype(BF16)
    ms = ((Wk.T @ bq) * SCALE).astype(BF16).reshape(1, D)
    ones = np.ones((1, 128), dtype=BF16)
    wv = Wv.sum(axis=0)                             # (D,)
    wvrep = np.ascontiguousarray(np.broadcast_to(wv.astype(BF16), (128, D)))
    c1 = float(bv.sum())

    nc = _build()

    in_maps = []
    for c in range(NCORES):
        Xc = pf[c * PB:(c + 1) * PB]                # (PB, 3, D)
        Xr = Xc.reshape(G, 128, 3, D).transpose(0, 2, 1, 3).reshape(NBLK, 128, D)
        xn = Xr.astype(BF16)
        xt = np.ascontiguousarray(
            Xr.reshape(NBLK, 128, DC, 128).transpose(0, 2, 3, 1)).astype(BF16)
        in_maps.append({"xt": xt, "xn": xn, "mm": mm, "mrow": ms,
                        "ones": ones, "wvrep": wvrep})

    trace = os.environ.get("KB_TRACE", "") == "1"
    res = run_bass_kernel_spmd(nc, in_maps, core_ids=list(range(NCORES)),
                               trace=trace)
    LAST_RESULTS = res

    out = np.concatenate(
        [res.results[c]["out"].reshape(PB, 3) for c in range(NCORES)], axis=0)
    return (out + c1).astype(np.float32)


# revision 11
# speedup vs baseline: 13485.4881x; 1.1545x over previous
"""PointerHead kernel for 8x TRN2 NeuronCores.

Reference computes, per batch row b with X = path_features[b] (3 x 2048):
    qkv = X @ W.T + b_             (3 x 6144), split into Q,K,V (3 x 2048)
    S   = Q @ K.T / sqrt(2048)     (3 x 3)
    A   = softmax(S, axis=-1)
    out = (A @ V).sum(-1)          (3,)

Algebraic reductions used here (all exact in real arithmetic):
  * S = X Ms X^T + 1 qs^T (+ row-const, dropped by softmax) with
    Ms = (Wq^T Wk)/sqrt(d), qs_j = X_j . (Wk^T bq)/sqrt(d).
    The qs term is folded into the matmul as one extra contraction row:
    Y = [X | 1] @ [[Ms], [ms]] with ms = (Wk^T bq)/sqrt(d), so
    S_ij = Y_i . X_j directly.
  * out_i = sum_j A_ij * (X_j . colsum(Wv)) + sum(bv); the constant is
    added on host (softmax rows sum to 1), V is never materialized.
Device work per core (1024 batches): one (3072x2048)@(2048x2048) bf16
matmul (Y) + DVE row-dot grams (tensor_mul + reduce_sum; the fused
tensor_tensor_reduce crashes the exec unit on this HW) + tiny softmax.
"""

import math
import os
import sys

import numpy as np

if "/opt/trn_rl_repo" not in sys.path:
    sys.path.insert(0, "/opt/trn_rl_repo")

import ml_dtypes

BF16 = ml_dtypes.bfloat16

D = 2048
B = 8192
NCORES = 8
PB = B // NCORES        # batches per core
G = PB // 128           # groups of 128 batches per core
NBLK = 3 * G            # row blocks per core; blk = g*3 + i
DC = D // 128           # contraction chunks
SCALE = 1.0 / math.sqrt(D)

_CACHE = {}
LAST_RESULTS = None


def _build():
    if "nc" in _CACHE:
        return _CACHE["nc"]

    import concourse.bass as bass
    import concourse.tile as tile
    from concourse import bacc, mybir

    bf = mybir.dt.bfloat16
    f32 = mybir.dt.float32

    nc = bacc.Bacc("TRN2", target_bir_lowering=False, debug=False,
                   num_devices=NCORES)

    xt_d = nc.dram_tensor("xt", [NBLK, 128, DC, 128], bf, kind="ExternalInput")
    xn_d = nc.dram_tensor("xn", [G, 128, 3, D], bf, kind="ExternalInput")
    mm_d = nc.dram_tensor("mm", [128, DC, D], bf, kind="ExternalInput")
    mr_d = nc.dram_tensor("mrow", [1, D], bf, kind="ExternalInput")
    on_d = nc.dram_tensor("ones", [1, 128], bf, kind="ExternalInput")
    wr_d = nc.dram_tensor("wvrep", [128, D], bf, kind="ExternalInput")
    out_d = nc.dram_tensor("out", [G, 128, 3], f32, kind="ExternalOutput")

    AX = mybir.AxisListType
    AF = mybir.ActivationFunctionType

    with tile.TileContext(nc) as tc:
        with (
            tc.tile_pool(name="wpool", bufs=1) as wpool,
            tc.tile_pool(name="xtp", bufs=3) as xtp,
            tc.tile_pool(name="xnp", bufs=2) as xnp,
            tc.tile_pool(name="ypool", bufs=2) as ypool,
            tc.tile_pool(name="prodp", bufs=2) as prodp,
            tc.tile_pool(name="small", bufs=3) as small,
            tc.tile_pool(name="psum", bufs=2, space="PSUM") as psum,
        ):
            mm_s = wpool.tile([128, DC, D], bf)
            nc.sync.dma_start(mm_s[:], mm_d[:])
            mr_s = wpool.tile([1, D], bf)
            on_s = wpool.tile([1, 128], bf)
            wr_s = wpool.tile([128, D], bf)
            nc.gpsimd.dma_start(mr_s[:], mr_d[:])
            nc.gpsimd.dma_start(on_s[:], on_d[:])
            nc.gpsimd.dma_start(wr_s[:], wr_d[:])

            for g in range(G):
                xn3 = xnp.tile([128, 3, D], bf)
                nc.gpsimd.dma_start(xn3[:], xn_d[g])

                # vs_j = X_j . colsum(Wv) for the 128 batches of this group
                vs = small.tile([128, 3], f32)
                prod = prodp.tile([128, D], bf)
                for j in range(3):
                    nc.vector.tensor_mul(prod[:], xn3[:, j, :], wr_s[:])
                    nc.vector.reduce_sum(vs[:, j:j + 1], prod[:], axis=AX.X)

                # Y_i = [X_i | 1] @ [[Ms],[ms]] on PE, grams on DVE
                y3 = ypool.tile([128, 3, D], bf)
                for i in range(3):
                    blk = g * 3 + i
                    xts = xtp.tile([128, DC, 128], bf)
                    nc.sync.dma_start(xts[:], xt_d[blk])
                    py = psum.tile([128, D], f32)
                    for dc in range(DC):
                        for ec in range(4):
                            nc.tensor.matmul(
                                py[:, ec * 512:(ec + 1) * 512],
                                xts[:, dc, :],
                                mm_s[:, dc, ec * 512:(ec + 1) * 512],
                                start=(dc == 0), stop=False)
                    for ec in range(4):
                        nc.tensor.matmul(
                            py[:, ec * 512:(ec + 1) * 512],
                            on_s[:],
                            mr_s[:, ec * 512:(ec + 1) * 512],
                            start=False, stop=True)
                    nc.scalar.copy(y3[:, i, :], py[:])

                # S[:, i*3+j] = Y_i . X_j  (scale already folded into Ms/ms)
                S = small.tile([128, 9], f32)
                for i in range(3):
                    for j in range(3):
                        nc.vector.tensor_mul(prod[:], y3[:, i, :], xn3[:, j, :])
                        nc.vector.reduce_sum(S[:, 3 * i + j:3 * i + j + 1],
                                             prod[:], axis=AX.X)

                E = small.tile([128, 9], f32)
                nc.scalar.activation(E[:], S[:], AF.Exp)

                den = small.tile([128, 3], f32)
                num = small.tile([128, 3], f32)
                prod3 = small.tile([128, 3], f32)
                for i in range(3):
                    nc.vector.reduce_sum(den[:, i:i + 1], E[:, 3 * i:3 * i + 3],
                                         axis=AX.X)
                    nc.vector.tensor_mul(prod3[:], E[:, 3 * i:3 * i + 3], vs[:])
                    nc.vector.reduce_sum(num[:, i:i + 1], prod3[:], axis=AX.X)
                rden = small.tile([128, 3], f32)
                nc.vector.reciprocal(rden[:], den[:])
                logit = small.tile([128, 3], f32)
                nc.vector.tensor_mul(logit[:], num[:], rden[:])
                nc.gpsimd.dma_start(out_d[g], logit[:])

    nc.compile()
    _CACHE["nc"] = nc
    return nc


def kernel(path_features, W, b):
    global LAST_RESULTS
    from concourse.bass_utils import run_bass_kernel_spmd

    pf = np.ascontiguousarray(path_features, dtype=np.float32)
    W = np.asarray(W, dtype=np.float32)
    bias = np.asarray(b, dtype=np.float32)
    Wq, Wk, Wv = W[0:D], W[D:2 * D], W[2 * D:3 * D]
    bq, bv = bias[0:D], bias[2 * D:3 * D]

    Ms = (Wq.T @ Wk) * SCALE                        # (D, D) fp32
    mm = np.ascontiguousarray(
        Ms.reshape(DC, 128, D).transpose(1, 0, 2)).astype(BF16)
    ms = ((Wk.T @ bq) * SCALE).astype(BF16).reshape(1, D)
    ones = np.ones((1, 128), dtype=BF16)
    wv = Wv.sum(axis=0)                             # (D,)
    wvrep = np.ascontiguousarray(np.broadcast_to(wv.astype(BF16), (128, D)))
    c1 = float(bv.sum())

    nc = _build()

    in_maps = []
    for c in range(NCORES):
        Xc = pf[c * PB:(c + 1) * PB]                # (PB, 3, D)
        xn = Xc.reshape(G, 128, 3, D).astype(BF16)
        Xr = Xc.reshape(G, 128, 3, D).transpose(0, 2, 1, 3).reshape(NBLK, 128, D)
        xt = np.ascontiguousarray(
            Xr.reshape(NBLK, 128, DC, 128).transpose(0, 3, 2, 1)).astype(BF16)
        in_maps.append({"xt": xt, "xn": xn, "mm": mm, "mrow": ms,
                        "ones": ones, "wvrep": wvrep})

    trace = os.environ.get("KB_TRACE", "") == "1"
    res = run_bass_kernel_spmd(nc, in_maps, core_ids=list(range(NCORES)),
                               trace=trace)
    LAST_RESULTS = res

    out = np.concatenate(
        [res.results[c]["out"].reshape(PB, 3) for c in range(NCORES)], axis=0)
    return (out + c1).astype(np.float32)


# revision 14
# speedup vs baseline: 14580.5504x; 1.0812x over previous
"""PointerHead kernel for 8x TRN2 NeuronCores.

Reference computes, per batch row b with X = path_features[b] (3 x 2048):
    qkv = X @ W.T + b_             (3 x 6144), split into Q,K,V (3 x 2048)
    S   = Q @ K.T / sqrt(2048)     (3 x 3)
    A   = softmax(S, axis=-1)
    out = (A @ V).sum(-1)          (3,)

Algebraic reductions used here (all exact in real arithmetic):
  * S_ij = X_i Ms X_j^T + qs_j (+ row-const, dropped by softmax) with
    Ms = (Wq^T Wk)/sqrt(d), qs_j = X_j . ms, ms = (Wk^T bq)/sqrt(d).
    The Gram part comes from one PE matmul Y_i = X_i @ Ms per row block,
    then DVE row-dots S_ij = Y_i . X_j; qs is added post-Gram on DVE.
  * out_i = sum_j A_ij * (X_j . colsum(Wv)) + sum(bv); the constant is
    added on host (softmax rows sum to 1), V is never materialized.
Device work per core (1024 batches): one (3072x2048)@(2048x2048) bf16
matmul + fused affine_mul_reduce row-dots (the plain tensor_tensor_reduce
opcode crashes the exec unit on this HW; the CUSTOM_DVE_ANT family works)
+ tiny softmax. mm is split over 4 DMA queues so PE starts early.
"""

import math
import os
import sys

import numpy as np

if "/opt/trn_rl_repo" not in sys.path:
    sys.path.insert(0, "/opt/trn_rl_repo")

import ml_dtypes

BF16 = ml_dtypes.bfloat16

D = 2048
B = 8192
NCORES = 8
PB = B // NCORES        # batches per core
G = PB // 128           # groups of 128 batches per core
NBLK = 3 * G            # row blocks per core; blk = g*3 + i
DC = D // 128           # contraction chunks
QD = DC // 4            # dc chunks per mm DMA quarter
SCALE = 1.0 / math.sqrt(D)

_CACHE = {}
LAST_RESULTS = None


def _build():
    if "nc" in _CACHE:
        return _CACHE["nc"]

    import concourse.bass as bass
    import concourse.tile as tile
    from concourse import bacc, mybir

    bf = mybir.dt.bfloat16
    f32 = mybir.dt.float32

    nc = bacc.Bacc("TRN2", target_bir_lowering=False, debug=False,
                   num_devices=NCORES)

    xt_d = nc.dram_tensor("xt", [NBLK, 128, DC, 128], bf, kind="ExternalInput")
    xn_d = nc.dram_tensor("xn", [G, 128, 3, D], bf, kind="ExternalInput")
    mm_d = nc.dram_tensor("mm", [128, DC, D], bf, kind="ExternalInput")
    ms_d = nc.dram_tensor("msrep", [128, D], bf, kind="ExternalInput")
    wr_d = nc.dram_tensor("wvrep", [128, D], bf, kind="ExternalInput")
    out_d = nc.dram_tensor("out", [G, 128, 3], f32, kind="ExternalOutput")

    AX = mybir.AxisListType
    AF = mybir.ActivationFunctionType

    with tile.TileContext(nc) as tc:
        with (
            tc.tile_pool(name="wpool", bufs=1) as wpool,
            tc.tile_pool(name="xtp", bufs=3) as xtp,
            tc.tile_pool(name="xnp", bufs=2) as xnp,
            tc.tile_pool(name="ypool", bufs=2) as ypool,
            tc.tile_pool(name="prodp", bufs=2) as prodp,
            tc.tile_pool(name="small", bufs=3) as small,
            tc.tile_pool(name="psum", bufs=2, space="PSUM") as psum,
        ):
            # mm quartered across 4 DMA queues: separate tiles give
            # per-quarter deps so block-0 matmuls start on quarter 0.
            mm_q = [wpool.tile([128, QD, D], bf, name=f"mmq{q}")
                    for q in range(4)]
            wr_s = wpool.tile([128, D], bf)
            ms_s = wpool.tile([128, D], bf)
            nc.sync.dma_start(wr_s[:], wr_d[:])
            nc.sync.dma_start(ms_s[:], ms_d[:])
            nc.scalar.dma_start(mm_q[0][:], mm_d[:, 0 * QD:1 * QD, :])
            nc.scalar.dma_start(mm_q[1][:], mm_d[:, 1 * QD:2 * QD, :])
            nc.gpsimd.dma_start(mm_q[2][:], mm_d[:, 2 * QD:3 * QD, :])
            nc.gpsimd.dma_start(mm_q[3][:], mm_d[:, 3 * QD:4 * QD, :])

            for g in range(G):
                xn3 = xnp.tile([128, 3, D], bf)
                nc.gpsimd.dma_start(xn3[:], xn_d[g])

                # vs_j = X_j . colsum(Wv), qs_j = X_j . ms  (fused dot)
                vs = small.tile([128, 3], f32)
                qs = small.tile([128, 3], f32)
                prod = prodp.tile([128, D], bf)
                for j in range(3):
                    nc.vector.affine_mul_reduce(
                        out=prod[:], accum_out=vs[:, j:j + 1],
                        in0=xn3[:, j, :], in1=wr_s[:], scale=1.0, bias=0.0)
                for j in range(3):
                    nc.vector.affine_mul_reduce(
                        out=prod[:], accum_out=qs[:, j:j + 1],
                        in0=xn3[:, j, :], in1=ms_s[:], scale=1.0, bias=0.0)

                # Y_i = X_i @ Ms on PE; Gram row S[:, 3i+j] = Y_i . X_j
                S = small.tile([128, 9], f32)
                for i in range(3):
                    blk = g * 3 + i
                    xts = xtp.tile([128, DC, 128], bf)
                    nc.sync.dma_start(xts[:], xt_d[blk])
                    py = psum.tile([128, D], f32)
                    for dc in range(DC):
                        mq = mm_q[dc // QD]
                        for ec in range(4):
                            nc.tensor.matmul(
                                py[:, ec * 512:(ec + 1) * 512],
                                xts[:, dc, :],
                                mq[:, dc % QD, ec * 512:(ec + 1) * 512],
                                start=(dc == 0), stop=(dc == DC - 1))
                    yb = ypool.tile([128, D], bf)
                    nc.scalar.copy(yb[:], py[:])
                    for j in range(3):
                        nc.vector.affine_mul_reduce(
                            out=prod[:], accum_out=S[:, 3 * i + j:3 * i + j + 1],
                            in0=yb[:], in1=xn3[:, j, :], scale=1.0, bias=0.0)

                # softmax over j with qs added; out_i = sum_j A_ij vs_j
                S2 = small.tile([128, 9], f32)
                for i in range(3):
                    nc.vector.tensor_add(S2[:, 3 * i:3 * i + 3],
                                         S[:, 3 * i:3 * i + 3], qs[:])
                E = small.tile([128, 9], f32)
                nc.scalar.activation(E[:], S2[:], AF.Exp)

                den = small.tile([128, 3], f32)
                num = small.tile([128, 3], f32)
                junk3 = small.tile([128, 3], f32)
                for i in range(3):
                    nc.vector.reduce_sum(den[:, i:i + 1], E[:, 3 * i:3 * i + 3],
                                         axis=AX.X)
                    nc.vector.affine_mul_reduce(
                        out=junk3[:], accum_out=num[:, i:i + 1],
                        in0=E[:, 3 * i:3 * i + 3], in1=vs[:],
                        scale=1.0, bias=0.0)
                rden = small.tile([128, 3], f32)
                nc.vector.reciprocal(rden[:], den[:])
                logit = small.tile([128, 3], f32)
                nc.vector.tensor_mul(logit[:], num[:], rden[:])
                nc.scalar.dma_start(out_d[g], logit[:])

    nc.compile()
    _CACHE["nc"] = nc
    return nc


def kernel(path_features, W, b):
    global LAST_RESULTS
    from concourse.bass_utils import run_bass_kernel_spmd

    pf = np.ascontiguousarray(path_features, dtype=np.float32)
    W = np.asarray(W, dtype=np.float32)
    bias = np.asarray(b, dtype=np.float32)
    Wq, Wk, Wv = W[0:D], W[D:2 * D], W[2 * D:3 * D]
    bq, bv = bias[0:D], bias[2 * D:3 * D]

    Ms = (Wq.T @ Wk) * SCALE                        # (D, D) fp32
    mm = np.ascontiguousarray(
        Ms.reshape(DC, 128, D).transpose(1, 0, 2)).astype(BF16)
    ms = ((Wk.T @ bq) * SCALE).astype(BF16)         # (D,)
    msrep = np.ascontiguousarray(np.broadcast_to(ms, (128, D)))
    wv = Wv.sum(axis=0)                             # (D,)
    wvrep = np.ascontiguousarray(np.broadcast_to(wv.astype(BF16), (128, D)))
    c1 = float(bv.sum())

    nc = _build()

    in_maps = []
    for c in range(NCORES):
        Xc = pf[c * PB:(c + 1) * PB]                # (PB, 3, D)
        xn = Xc.reshape(G, 128, 3, D).astype(BF16)
        Xr = Xc.reshape(G, 128, 3, D).transpose(0, 2, 1, 3).reshape(NBLK, 128, D)
        xt = np.ascontiguousarray(
            Xr.reshape(NBLK, 128, DC, 128).transpose(0, 3, 2, 1)).astype(BF16)
        in_maps.append({"xt": xt, "xn": xn, "mm": mm, "msrep": msrep,
                        "wvrep": wvrep})

    trace = os.environ.get("KB_TRACE", "") == "1"
    res = run_bass_kernel_spmd(nc, in_maps, core_ids=list(range(NCORES)),
                               trace=trace)
    LAST_RESULTS = res

    out = np.concatenate(
        [res.results[c]["out"].reshape(PB, 3) for c in range(NCORES)], axis=0)
    return (out + c1).astype(np.float32)


# revision 20
# speedup vs baseline: 14678.5654x; 1.0067x over previous
"""PointerHead kernel for 8x TRN2 NeuronCores.

Reference computes, per batch row b with X = path_features[b] (3 x 2048):
    qkv = X @ W.T + b_             (3 x 6144), split into Q,K,V (3 x 2048)
    S   = Q @ K.T / sqrt(2048)     (3 x 3)
    A   = softmax(S, axis=-1)
    out = (A @ V).sum(-1)          (3,)

Algebraic reductions used here (all exact in real arithmetic):
  * S_ij = X_i Ms X_j^T + qs_j (+ row-const, dropped by softmax) with
    Ms = (Wq^T Wk)/sqrt(d), qs_j = X_j . ms, ms = (Wk^T bq)/sqrt(d).
    The Gram part comes from one PE matmul Y_i = X_i @ Ms per row block,
    then DVE row-dots S_ij = Y_i . X_j; qs is added post-Gram on DVE.
  * out_i = sum_j A_ij * (X_j . colsum(Wv)) + sum(bv); the constant is
    added on host (softmax rows sum to 1), V is never materialized.
Device work per core (1024 batches): one (3072x2048)@(2048x2048) bf16
matmul + fused affine_mul_reduce row-dots (the plain tensor_tensor_reduce
opcode crashes the exec unit on this HW; the CUSTOM_DVE_ANT family works)
+ tiny softmax. mm is split over 4 DMA queues so PE starts early.
"""

import math
import os
import sys

import numpy as np

if "/opt/trn_rl_repo" not in sys.path:
    sys.path.insert(0, "/opt/trn_rl_repo")

import ml_dtypes

BF16 = ml_dtypes.bfloat16

D = 2048
B = 8192
NCORES = 8
PB = B // NCORES        # batches per core
G = PB // 128           # groups of 128 batches per core
NBLK = 3 * G            # row blocks per core; blk = g*3 + i
DC = D // 128           # contraction chunks
SCALE = 1.0 / math.sqrt(D)

_CACHE = {}
LAST_RESULTS = None


def _build():
    if "nc" in _CACHE:
        return _CACHE["nc"]

    import concourse.bass as bass
    import concourse.tile as tile
    from concourse import bacc, mybir

    bf = mybir.dt.bfloat16
    f32 = mybir.dt.float32

    nc = bacc.Bacc("TRN2", target_bir_lowering=False, debug=False,
                   num_devices=NCORES)

    xt_d = nc.dram_tensor("xt", [NBLK, 128, DC, 128], bf, kind="ExternalInput")
    xn_d = nc.dram_tensor("xn", [G, 128, 3, D], bf, kind="ExternalInput")
    mm_d = nc.dram_tensor("mm", [DC, 128, D], bf, kind="ExternalInput")
    ms_d = nc.dram_tensor("msrep", [128, D], bf, kind="ExternalInput")
    wr_d = nc.dram_tensor("wvrep", [128, D], bf, kind="ExternalInput")
    out_d = nc.dram_tensor("out", [G, 128, 3], f32, kind="ExternalOutput")

    AX = mybir.AxisListType
    AF = mybir.ActivationFunctionType

    with tile.TileContext(nc) as tc:
        with (
            tc.tile_pool(name="wpool", bufs=1) as wpool,
            tc.tile_pool(name="xtp", bufs=3) as xtp,
            tc.tile_pool(name="xnp", bufs=2) as xnp,
            tc.tile_pool(name="vqp", bufs=2) as vqp,
            tc.tile_pool(name="ypool", bufs=2) as ypool,
            tc.tile_pool(name="prodp", bufs=2) as prodp,
            tc.tile_pool(name="small", bufs=3) as small,
            tc.tile_pool(name="psum", bufs=2, space="PSUM") as psum,
        ):
            # mm as 16 per-dc tiles so block-0 matmuls start on piece 0;
            # pieces alternate scalar/gpsimd DMA rings to land sooner.
            wr_s = wpool.tile([128, D], bf)
            ms_s = wpool.tile([128, D], bf)
            nc.sync.dma_start(wr_s[:], wr_d[:])
            nc.sync.dma_start(ms_s[:], ms_d[:])
            xn3_c = xnp.tile([128, 3, D], bf, name="xn3")
            nc.gpsimd.dma_start(xn3_c[:], xn_d[0])
            mm_c = [wpool.tile([128, D], bf, name=f"mmc{dc}")
                    for dc in range(DC)]
            for dc in range(DC):
                eng = nc.scalar if dc % 2 == 0 else nc.gpsimd
                eng.dma_start(mm_c[dc][:], mm_d[dc])

            def vsqs(xn3):
                vs = vqp.tile([128, 3], f32, name="vs")
                qs = vqp.tile([128, 3], f32, name="qs")
                prod = prodp.tile([128, D], bf, name="prodw")
                for j in range(3):
                    nc.vector.affine_mul_reduce(
                        out=prod[:], accum_out=vs[:, j:j + 1],
                        in0=xn3[:, j, :], in1=wr_s[:], scale=1.0, bias=0.0)
                for j in range(3):
                    nc.vector.affine_mul_reduce(
                        out=prod[:], accum_out=qs[:, j:j + 1],
                        in0=xn3[:, j, :], in1=ms_s[:], scale=1.0, bias=0.0)
                return vs, qs

            # vs_j = X_j . colsum(Wv), qs_j = X_j . ms — pipelined one
            # group ahead so the DVE fills its wait-for-Y windows.
            vs_c, qs_c = vsqs(xn3_c)

            for g in range(G):
                if g + 1 < G:
                    xn3_n = xnp.tile([128, 3, D], bf, name="xn3")
                    nc.gpsimd.dma_start(xn3_n[:], xn_d[g + 1])

                # Y_i = X_i @ Ms on PE; Gram row S[:, 3i+j] = Y_i . X_j
                S = small.tile([128, 9], f32)
                prod = prodp.tile([128, D], bf)
                for i in range(3):
                    blk = g * 3 + i
                    xts = xtp.tile([128, DC, 128], bf)
                    nc.sync.dma_start(xts[:], xt_d[blk])
                    py = psum.tile([128, D], f32)
                    for dc in range(DC):
                        for ec in range(4):
                            nc.tensor.matmul(
                                py[:, ec * 512:(ec + 1) * 512],
                                xts[:, dc, :],
                                mm_c[dc][:, ec * 512:(ec + 1) * 512],
                                start=(dc == 0), stop=(dc == DC - 1))
                    yb = ypool.tile([128, D], bf)
                    nc.scalar.copy(yb[:], py[:])
                    for j in range(3):
                        nc.vector.affine_mul_reduce(
                            out=prod[:], accum_out=S[:, 3 * i + j:3 * i + j + 1],
                            in0=yb[:], in1=xn3_c[:, j, :], scale=1.0, bias=0.0)

                # softmax over j with qs added; out_i = sum_j A_ij vs_j
                S2 = small.tile([128, 9], f32)
                for i in range(3):
                    nc.vector.tensor_add(S2[:, 3 * i:3 * i + 3],
                                         S[:, 3 * i:3 * i + 3], qs_c[:])
                E = small.tile([128, 9], f32)
                nc.scalar.activation(E[:], S2[:], AF.Exp)

                den = small.tile([128, 3], f32)
                num = small.tile([128, 3], f32)
                junk3 = small.tile([128, 3], f32)
                for i in range(3):
                    nc.vector.reduce_sum(den[:, i:i + 1], E[:, 3 * i:3 * i + 3],
                                         axis=AX.X)
                    nc.vector.affine_mul_reduce(
                        out=junk3[:], accum_out=num[:, i:i + 1],
                        in0=E[:, 3 * i:3 * i + 3], in1=vs_c[:],
                        scale=1.0, bias=0.0)
                rden = small.tile([128, 3], f32)
                nc.vector.reciprocal(rden[:], den[:])
                logit = small.tile([128, 3], f32)
                nc.vector.tensor_mul(logit[:], num[:], rden[:])
                nc.scalar.dma_start(out_d[g], logit[:])

                if g + 1 < G:
                    vs_c, qs_c = vsqs(xn3_n)
                    xn3_c = xn3_n

    nc.compile()
    _CACHE["nc"] = nc
    return nc


def kernel(path_features, W, b):
    global LAST_RESULTS
    from concourse.bass_utils import run_bass_kernel_spmd

    pf = np.ascontiguousarray(path_features, dtype=np.float32)
    W = np.asarray(W, dtype=np.float32)
    bias = np.asarray(b, dtype=np.float32)
    Wq, Wk, Wv = W[0:D], W[D:2 * D], W[2 * D:3 * D]
    bq, bv = bias[0:D], bias[2 * D:3 * D]

    Ms = (Wq.T @ Wk) * SCALE                        # (D, D) fp32
    mm = np.ascontiguousarray(Ms.reshape(DC, 128, D)).astype(BF16)
    ms = ((Wk.T @ bq) * SCALE).astype(BF16)         # (D,)
    msrep = np.ascontiguousarray(np.broadcast_to(ms, (128, D)))
    wv = Wv.sum(axis=0)                             # (D,)
    wvrep = np.ascontiguousarray(np.broadcast_to(wv.astype(BF16), (128, D)))
    c1 = float(bv.sum())

    nc = _build()

    in_maps = []
    for c in range(NCORES):
        Xc = pf[c * PB:(c + 1) * PB]                # (PB, 3, D)
        xn = Xc.reshape(G, 128, 3, D).astype(BF16)
        Xr = Xc.reshape(G, 128, 3, D).transpose(0, 2, 1, 3).reshape(NBLK, 128, D)
        xt = np.ascontiguousarray(
            Xr.reshape(NBLK, 128, DC, 128).transpose(0, 3, 2, 1)).astype(BF16)
        in_maps.append({"xt": xt, "xn": xn, "mm": mm, "msrep": msrep,
                        "wvrep": wvrep})

    trace = os.environ.get("KB_TRACE", "") == "1"
    res = run_bass_kernel_spmd(nc, in_maps, core_ids=list(range(NCORES)),
                               trace=trace)
    LAST_RESULTS = res

    out = np.concatenate(
        [res.results[c]["out"].reshape(PB, 3) for c in range(NCORES)], axis=0)
    return (out + c1).astype(np.float32)
